# revision 1
# baseline (speedup 1.0000x reference)
"""Chamfer loss (two 16384x16384 1-NN searches + gathered MSE) on 8 Trainium2 cores.

Device (per core, queries sharded 8-way across cores):
  - PE: score matrix S[i,j] = q_i . r_j - |r_j|^2/2 - |q_i|^2/2 = -d(i,j)/2 via
    K=5 augmented fp32r matmuls ([5,128] x [5,512] -> PSUM), 32 j-tiles per
    128-query block. argmax_j S = argmin_j dist. This is >99.7% of the FLOPs.
  - ScalarE drains PSUM quads to an fp16 score row (Sfull [128, 16384]).
  - DVE folds Sfull twice by elementwise max (8192 -> 4096 slots; slot v covers
    candidates v + {0, 4096, 8192, 12288}), then max/max_index extract the
    top-8 fp16 slots per query -> 32 candidate indices covering the true
    argmin with a wide margin over fp32r/fp16 rounding noise (worst-case
    crowding at that noise window is ~23 candidates, measured offline).

Host:
  - Exact fp32 re-scoring of the 32 candidates per query (the same numpy-fp32
    formula as the reference), first-index argmax -> exact 1-NN index.
  - Gather e rows, squared-error means in f64 -> final f32 scalar.

Per-row indirect-DMA gathers on TRN2 cost ~1us/row of descriptor generation
(measured ~100ms for the on-device refinement variant), so the tiny
refinement lives on the host instead.
"""
import sys

sys.path.insert(0, "/opt/trn_rl_repo")

import numpy as np

import concourse.bass as bass
import concourse.bacc as bacc
import concourse.mybir as mybir
from concourse.tile import TileContext
from concourse.bass_utils import run_bass_kernel_spmd

P = 128          # partitions / queries per block
V = 16384        # reference points
NCORES = 8
QPC = V // NCORES            # queries per core per direction (2048)
NBLK = QPC // P              # query blocks per direction (16)
NQUAD = 8                    # psum quads of 4 j-tiles (4 x 512 cols)
NCAND = 32                   # candidates per query: top-8 slots x 4
ACT_QUADS = 8                # PSUM->SBUF drain quads on ScalarE (rest: DVE)
F16 = mybir.dt.float16
F32 = mybir.dt.float32
F32R = mybir.dt.float32r
U32 = mybir.dt.uint32

_CACHE = {}


def build(n_blocks=NBLK):
    nc = bacc.Bacc()
    qT = nc.dram_tensor("qT", [5, 2 * QPC], F32, kind="ExternalInput")
    rT = nc.dram_tensor("rT", [5, 2 * V], F32, kind="ExternalInput")
    slot_out = nc.dram_tensor(
        "slot_out", [P, 2 * n_blocks * 8], U32, kind="ExternalOutput"
    )

    with TileContext(nc) as tc:
        with (
            tc.tile_pool(name="tab", bufs=1) as tab,
            tc.tile_pool(name="sb", bufs=2) as sb,
            tc.tile_pool(name="fold", bufs=1) as fold,
            tc.tile_pool(name="small", bufs=3) as sm,
            tc.tile_pool(name="acc", bufs=1) as accp,
            tc.tile_pool(name="ps", bufs=2, space="PSUM") as ps,
        ):
            qt = tab.tile([5, 2 * QPC], F32R)
            nc.sync.dma_start(out=qt[:], in_=qT[:].bitcast(F32R))
            slotbuf = accp.tile([P, 2 * n_blocks * 8], U32)

            for d in range(2):
                rt = tab.tile([5, V], F32R, tag="rt")
                nc.sync.dma_start(
                    out=rt[:], in_=rT[:, d * V : (d + 1) * V].bitcast(F32R)
                )
                for b in range(n_blocks):
                    lhsT = qt[:, d * QPC + b * P : d * QPC + (b + 1) * P]
                    sfull = sb.tile([P, V], F16, tag="sfull")
                    for g in range(NQUAD):
                        psq = ps.tile([P, 2048], F32, tag="psq")
                        for t in range(4):
                            nc.tensor.matmul(
                                out=psq[:, t * 512 : (t + 1) * 512],
                                lhsT=lhsT,
                                rhs=rt[:, (g * 4 + t) * 512 : (g * 4 + t + 1) * 512],
                                start=True,
                                stop=True,
                            )
                        dst = sfull[:, g * 2048 : (g + 1) * 2048]
                        if g < ACT_QUADS:
                            nc.scalar.copy(dst, psq[:])
                        else:
                            nc.vector.tensor_copy(dst, psq[:])
                    spair = fold.tile([P, 8192], F16, tag="spair")
                    nc.vector.tensor_tensor(
                        out=spair[:],
                        in0=sfull[:, 0:8192],
                        in1=sfull[:, 8192:V],
                        op=mybir.AluOpType.max,
                    )
                    squad = fold.tile([P, 4096], F16, tag="squad")
                    nc.vector.tensor_tensor(
                        out=squad[:],
                        in0=spair[:, 0:4096],
                        in1=spair[:, 4096:8192],
                        op=mybir.AluOpType.max,
                    )
                    m8 = sm.tile([P, 8], F16, tag="m8")
                    col = (d * n_blocks + b) * 8
                    nc.vector.max(out=m8[:], in_=squad[:])
                    nc.vector.max_index(
                        out=slotbuf[:, col : col + 8], in_max=m8[:], in_values=squad[:]
                    )
            nc.sync.dma_start(out=slot_out[:], in_=slotbuf[:])
    nc.compile()
    return nc


def _aug_tables(pred_vertices, trg_vertices):
    pv = np.ascontiguousarray(pred_vertices[0])  # [V,3]
    tv = np.ascontiguousarray(trg_vertices[0])

    def aug_ref_T(r):  # [5, V]: x, y, z, -|r|^2/2, 1
        n2 = ((r * r).sum(1) * np.float32(0.5)).astype(np.float32)
        return np.concatenate(
            [r.T, -n2[None, :], np.ones((1, V), np.float32)], axis=0
        )

    def aug_q_T(q):  # [5, Vq]: x, y, z, 1, -|q|^2/2
        n2 = ((q * q).sum(1) * np.float32(0.5)).astype(np.float32)
        return np.concatenate(
            [q.T, np.ones((1, q.shape[0]), np.float32), -n2[None, :]], axis=0
        )

    rT = np.ascontiguousarray(np.concatenate([aug_ref_T(pv), aug_ref_T(tv)], axis=1))
    qT_A, qT_B = aug_q_T(tv), aug_q_T(pv)
    return pv, tv, rT, qT_A, qT_B


def _prep_inputs(pred_vertices, trg_vertices, pred_e=None, trg_e=None):
    _, _, rT, qT_A, qT_B = _aug_tables(pred_vertices, trg_vertices)
    in_maps = []
    for c in range(NCORES):
        sl = slice(c * QPC, (c + 1) * QPC)
        in_maps.append(
            {
                "qT": np.ascontiguousarray(
                    np.concatenate([qT_A[:, sl], qT_B[:, sl]], axis=1)
                ),
                "rT": rT,
            }
        )
    return in_maps


def run_device(in_maps):
    if "nc" not in _CACHE:
        _CACHE["nc"] = build()
    return run_bass_kernel_spmd(_CACHE["nc"], in_maps, list(range(NCORES))).results


_OFFS = np.array([0, 4096, 8192, 12288], dtype=np.int64)


def _exact_indices(results, pv, tv):
    """Top-8 fp16 slots -> 32 candidates -> exact fp32 first-index argmax."""
    out = []
    for d, (q, r) in enumerate([(tv, pv), (pv, tv)]):
        slots = np.empty((V, 8), np.int64)
        for c in range(NCORES):
            so = results[c]["slot_out"]  # [P, 2*NBLK*8]
            for b in range(NBLK):
                rows = slice(c * QPC + b * P, c * QPC + (b + 1) * P)
                slots[rows] = so[:, (d * NBLK + b) * 8 : (d * NBLK + b + 1) * 8]
        cand = (slots[:, :, None] + _OFFS[None, None, :]).reshape(V, NCAND)
        n2 = ((r * r).sum(1) * np.float32(0.5)).astype(np.float32)
        rc = r[cand]                       # [V, 32, 3]
        s = np.einsum("vkc,vc->vk", rc, q).astype(np.float32) - n2[cand]
        smax = s.max(axis=1)
        masked = np.where(s >= smax[:, None], cand, 1 << 30)
        out.append(masked.min(axis=1))
    return out  # [idxA, idxB]


def kernel(pred_vertices, trg_vertices, pred_e, trg_e):
    pv, tv, _, _, _ = _aug_tables(pred_vertices, trg_vertices)
    in_maps = _prep_inputs(pred_vertices, trg_vertices)
    results = run_device(in_maps)
    idxA, idxB = _exact_indices(results, pv, tv)
    pe = np.ascontiguousarray(pred_e[0])
    te = np.ascontiguousarray(trg_e[0])
    lossA = ((te.astype(np.float64) - pe[idxA].astype(np.float64)) ** 2).sum() / (
        V * 3
    )
    lossB = ((pe.astype(np.float64) - te[idxB].astype(np.float64)) ** 2).sum() / (
        V * 3
    )
    return np.float32(lossA + lossB)


def kernel_indices(pred_vertices, trg_vertices, pred_e=None, trg_e=None):
    pv, tv, _, _, _ = _aug_tables(pred_vertices, trg_vertices)
    in_maps = _prep_inputs(pred_vertices, trg_vertices)
    results = run_device(in_maps)
    return _exact_indices(results, pv, tv)



# revision 7
# speedup vs baseline: 8.0939x; 8.0939x over previous
"""Chamfer loss (two 16384x16384 1-NN searches + gathered MSE) on 8 Trainium2
cores.

Device (per core; queries sharded 8-way, 2048 per core per direction):
  - PE: score S[i,j] = q_i . r_j - |r_j|^2/2 = -(d(i,j) - |q_i|^2)/2 via ONE
    K=11 fp16 matmul per 512-col tile: augmented rows [qh4, qh4, ql3] x
    [rh4, rl4, rh3] implement the hi/lo split qh.rh + qh.rl + ql.rh, so the
    fp16 input rounding error (~3e-5) stays below the quantizer step.
    argmax_j S = argmin_j dist.
  - DVE tensor_reduce (max) folds each PSUM quad [128,2048] -> 128 slots
    directly (slot = j>>4 covers 16 consecutive j), so the fp32 scores never
    round through fp16 and no separate PSUM drain is needed.
  - ScalarE quantizes the folded row acc[1024] to int32: qi = rint(acc*4096).
  - Pool packs keys K = qi*32 + (slot&31); |K| <= 12M < 2^24 keeps every
    int32 op exact even through float ALU paths.
  - DVE reduces K per 32-slot class -> 32 winners/query/block; low 5 bits
    decode the winning slot. No InstMaxIndex anywhere (it costs ~256us/call
    on HW, ~8ms/core total in the previous version - the old bottleneck).

Host:
  - Decode 32 winner slots -> 32*16 = 512 candidate ids per query; exact f64
    re-scoring picks the true 1-NN (first-index tie-break). Measured on the
    fixed harness inputs: 19/32768 flips vs the fp32 reference argmin,
    loss rel-err 5.4e-05 (gate is 2e-2).
  - Gather e rows, squared-error means in f64 -> final f32 scalar.

Dispatch: run_bass_kernel_spmd's axon path rebuilds jax.jit(shard_map(...))
on every call, which re-runs neuronx_cc_hook -> bir_verify_and_optimise
(~300-450ms of client-side Python per call, scaling with instruction count).
_Runner builds the identical _bass_exec_p/shard_map wrapper once and caches
it, so steady-state calls are transfer + execute only.
"""
import sys

sys.path.insert(0, "/opt/trn_rl_repo")

import numpy as np

import concourse.bass as bass
import concourse.bacc as bacc
import concourse.mybir as mybir
from concourse.tile import TileContext

P = 128          # partitions / queries per block
V = 16384        # reference points
NCORES = 8
QPC = V // NCORES            # queries per core per direction (2048)
NBLK = QPC // P              # query blocks per core per direction (16)
NQUAD = 8                    # PSUM quads of 4 x 512 cols per block
KDIM = 11                    # augmented contraction: qh4+qh4+ql3
GRP = 16                     # j's per slot (slot = j >> 4)
NSLOT = V // GRP             # 1024 slots
LOC = 32                     # slots per class
NCLS = NSLOT // LOC          # 32 classes -> 32 winners/query/block
QSCALE = 4096.0              # quantizer: qi = rint(acc * 4096)
F16 = mybir.dt.float16
F32 = mybir.dt.float32
I32 = mybir.dt.int32
AX = mybir.AxisListType
OP = mybir.AluOpType

_CACHE = {}


def build(n_blocks=NBLK):
    nc = bacc.Bacc()
    qT = nc.dram_tensor("qT", [KDIM, 2 * QPC], F16, kind="ExternalInput")
    rT = nc.dram_tensor("rT", [KDIM, 2 * V], F16, kind="ExternalInput")
    # slot_out keeps the full-width shape for every n_blocks so that the
    # timing comparator (n_blocks=1) has identical host<->device transfers.
    slot_out = nc.dram_tensor(
        "slot_out", [P, 2 * NBLK * NCLS], I32, kind="ExternalOutput"
    )

    with TileContext(nc) as tc:
        with (
            tc.tile_pool(name="tab", bufs=1) as tab,
            tc.tile_pool(name="rtp", bufs=2) as rtp,
            tc.tile_pool(name="sb", bufs=2) as sb,
            tc.tile_pool(name="pk", bufs=2) as pk,
            tc.tile_pool(name="acc", bufs=1) as accp,
            tc.tile_pool(name="ps", bufs=2, space="PSUM") as ps,
        ):
            slotbuf = accp.tile([P, 2 * NBLK * NCLS], I32)
            if n_blocks < NBLK:
                nc.vector.memset(slotbuf[:], 0)

            qt = tab.tile([KDIM, 2 * QPC], F16)
            nc.sync.dma_start(out=qt[:], in_=qT[:])

            iota_l = tab.tile([P, NSLOT], I32)
            nc.gpsimd.iota(
                iota_l[:], pattern=[[0, NCLS], [1, LOC]], base=0,
                channel_multiplier=0,
            )


            for d in range(2):
                rt = rtp.tile([KDIM, V], F16, tag="rt")
                nc.sync.dma_start(out=rt[:], in_=rT[:, d * V : (d + 1) * V])
                for b in range(n_blocks):
                    lhsT = qt[:, d * QPC + b * P : d * QPC + (b + 1) * P]
                    acc = sb.tile([P, NSLOT], F32, tag="acc")
                    for g in range(NQUAD):
                        psq = ps.tile([P, 2048], F32, tag="psq")
                        for t in range(4):
                            nc.tensor.matmul(
                                out=psq[:, t * 512 : (t + 1) * 512],
                                lhsT=lhsT,
                                rhs=rt[:, (g * 4 + t) * 512 : (g * 4 + t + 1) * 512],
                                start=True,
                                stop=True,
                            )
                        nc.vector.tensor_reduce(
                            out=acc[:, g * P : (g + 1) * P],
                            in_=psq[:].rearrange("p (s g) -> p s g", g=GRP),
                            axis=AX.X,
                            op=OP.max,
                        )
                    qi = pk.tile([P, NSLOT], I32, tag="qi")
                    nc.scalar.activation(
                        out=qi[:], in_=acc[:],
                        func=mybir.ActivationFunctionType.Copy, scale=QSCALE,
                    )
                    kq = pk.tile([P, NSLOT], I32, tag="kq")
                    nc.gpsimd.tensor_scalar(
                        out=kq[:], in0=qi[:], scalar1=float(LOC), scalar2=None,
                        op0=OP.mult,
                    )
                    kk = pk.tile([P, NSLOT], I32, tag="kk")
                    nc.gpsimd.tensor_tensor(
                        out=kk[:], in0=kq[:], in1=iota_l[:], op=OP.add
                    )
                    col = (d * n_blocks + b) * NCLS
                    nc.vector.tensor_reduce(
                        out=slotbuf[:, col : col + NCLS],
                        in_=kk[:].rearrange("p (c l) -> p c l", l=LOC),
                        axis=AX.X,
                        op=OP.max,
                    )
            nc.sync.dma_start(out=slot_out[:], in_=slotbuf[:])
    nc.compile()
    return nc


def _hilo(x):
    h = x.astype(np.float16)
    l = (x - h.astype(np.float32)).astype(np.float16)
    return h, l


def _aug_tables(pred_vertices, trg_vertices):
    pv = np.ascontiguousarray(pred_vertices[0]).astype(np.float32)  # [V,3]
    tv = np.ascontiguousarray(trg_vertices[0]).astype(np.float32)

    def aug_q_T(q):  # [11, Vq] fp16: [qh4, qh4, ql3]
        n = q.shape[0]
        qa = np.concatenate([q.T, np.ones((1, n), np.float32)], axis=0)  # [4,n]
        qh, ql = _hilo(qa)
        return np.concatenate([qh, qh, ql[:3]], axis=0)

    def aug_r_T(r):  # [11, V] fp16: [rh4, rl4, rh3]
        n2 = ((r * r).sum(1) * np.float32(0.5)).astype(np.float32)
        ra = np.concatenate([r.T, -n2[None, :]], axis=0)  # [4,V]
        rh, rl = _hilo(ra)
        return np.concatenate([rh, rl, rh[:3]], axis=0)

    # direction A: queries=tv, refs=pv;  direction B: queries=pv, refs=tv
    rT = np.ascontiguousarray(
        np.concatenate([aug_r_T(pv), aug_r_T(tv)], axis=1)
    )
    qT_A, qT_B = aug_q_T(tv), aug_q_T(pv)
    return pv, tv, rT, qT_A, qT_B


def _prep_inputs(pred_vertices, trg_vertices, pred_e=None, trg_e=None):
    _, _, rT, qT_A, qT_B = _aug_tables(pred_vertices, trg_vertices)
    in_maps = []
    for c in range(NCORES):
        sl = slice(c * QPC, (c + 1) * QPC)
        in_maps.append(
            {
                "qT": np.ascontiguousarray(
                    np.concatenate([qT_A[:, sl], qT_B[:, sl]], axis=1)
                ),
                "rT": rT,
            }
        )
    return in_maps


class _Runner:
    """Cached jit(shard_map(bass_exec)) dispatch for one compiled module.

    Mirrors run_bass_kernel_spmd's axon branch (bass2jax.run_bass_via_pjrt)
    but constructs the jitted callable once, so repeat calls skip the
    client-side re-trace / neuronx_cc_hook / BIR re-verification.
    """

    def __init__(self, nc):
        import jax
        from jax.sharding import Mesh, PartitionSpec
        from jax.experimental.shard_map import shard_map
        from concourse.bass2jax import (
            _bass_exec_p,
            install_neuronx_cc_hook,
            partition_id_tensor,
        )

        install_neuronx_cc_hook()
        partition_name = (
            nc.partition_id_tensor.name if nc.partition_id_tensor else None
        )
        in_names, out_names, out_avals, zero_shapes = [], [], [], []
        for alloc in nc.m.functions[0].allocations:
            if not isinstance(alloc, mybir.MemoryLocationSet):
                continue
            name = alloc.memorylocations[0].name
            if alloc.kind == "ExternalInput":
                if name != partition_name:
                    in_names.append(name)
            elif alloc.kind == "ExternalOutput":
                out_names.append(name)
                shape = tuple(alloc.tensor_shape)
                dtype = mybir.dt.np(alloc.dtype)
                out_avals.append(jax.core.ShapedArray(shape, dtype))
                zero_shapes.append((shape, dtype))
        n_params = len(in_names)
        all_names = list(in_names) + list(out_names)
        if partition_name is not None:
            all_names.append(partition_name)

        def _body(*args):
            operands = list(args)
            if partition_name is not None:
                operands.append(partition_id_tensor())
            outs = _bass_exec_p.bind(
                *operands,
                out_avals=tuple(out_avals),
                in_names=tuple(all_names),
                out_names=tuple(out_names),
                lowering_input_output_aliases=(),
                sim_require_finite=True,
                sim_require_nnan=True,
                nc=nc,
            )
            return tuple(outs)

        donate = tuple(range(n_params, n_params + len(out_avals)))
        devices = jax.devices()[:NCORES]
        mesh = Mesh(np.asarray(devices), ("core",))
        in_specs = (PartitionSpec("core"),) * (n_params + len(out_avals))
        out_specs = (PartitionSpec("core"),) * len(out_names)
        self._fn = jax.jit(
            shard_map(
                _body, mesh=mesh, in_specs=in_specs, out_specs=out_specs,
                check_rep=False,
            ),
            donate_argnums=donate,
            keep_unused=True,
        )
        self._in_names = in_names
        self._out_names = out_names
        self._out_avals = out_avals
        self._zero_shapes = zero_shapes

    def __call__(self, in_maps):
        concat_in = [
            np.concatenate([np.asarray(m[n]) for m in in_maps], axis=0)
            for n in self._in_names
        ]
        concat_zeros = [
            np.zeros((NCORES * s[0], *s[1:]), dt) for s, dt in self._zero_shapes
        ]
        out_arrs = self._fn(*concat_in, *concat_zeros)
        out_arrs = [np.asarray(o) for o in out_arrs]
        return [
            {
                name: out_arrs[i].reshape(NCORES, *self._out_avals[i].shape)[c]
                for i, name in enumerate(self._out_names)
            }
            for c in range(NCORES)
        ]


def get_runner(key="full", **build_kwargs):
    if key not in _CACHE:
        _CACHE[key] = _Runner(build(**build_kwargs))
    return _CACHE[key]


def run_device(in_maps):
    return get_runner()(in_maps)


_LOFF = np.arange(GRP, dtype=np.int64)


def _exact_indices(results, pv, tv):
    """Per-class winner keys -> 512 candidates/query -> exact f64 1-NN."""
    out = []
    cls_base = (np.arange(NCLS, dtype=np.int64) * LOC)[None, :]
    for d, (q, r) in enumerate([(tv, pv), (pv, tv)]):
        wins = np.empty((V, NCLS), np.int64)
        for c in range(NCORES):
            so = results[c]["slot_out"]  # [P, 2*NBLK*NCLS] int32
            for b in range(NBLK):
                rows = slice(c * QPC + b * P, c * QPC + (b + 1) * P)
                col = (d * NBLK + b) * NCLS
                wins[rows] = so[:, col : col + NCLS]
        l = np.mod(wins, LOC)                      # floor-mod: l even for K<0
        slots = cls_base + l                       # [V, NCLS]
        cand = (slots[:, :, None] * GRP + _LOFF[None, None, :]).reshape(
            V, NCLS * GRP
        )
        q64 = q.astype(np.float64)
        r64 = r.astype(np.float64)
        idx = np.empty(V, np.int64)
        CH = 4096
        for s0 in range(0, V, CH):
            sl = slice(s0, s0 + CH)
            rc = r64[cand[sl]]                     # [CH,512,3]
            dd = ((rc - q64[sl][:, None, :]) ** 2).sum(axis=2)
            dmin = dd.min(axis=1)
            masked = np.where(dd <= dmin[:, None], cand[sl], 1 << 40)
            idx[sl] = masked.min(axis=1)
        out.append(idx)
    return out  # [idxA, idxB]


def kernel(pred_vertices, trg_vertices, pred_e, trg_e):
    pv, tv, _, _, _ = _aug_tables(pred_vertices, trg_vertices)
    in_maps = _prep_inputs(pred_vertices, trg_vertices)
    results = run_device(in_maps)
    idxA, idxB = _exact_indices(results, pv, tv)
    pe = np.ascontiguousarray(pred_e[0])
    te = np.ascontiguousarray(trg_e[0])
    lossA = ((te.astype(np.float64) - pe[idxA].astype(np.float64)) ** 2).sum() / (
        V * 3
    )
    lossB = ((pe.astype(np.float64) - te[idxB].astype(np.float64)) ** 2).sum() / (
        V * 3
    )
    return np.float32(lossA + lossB)


def kernel_indices(pred_vertices, trg_vertices, pred_e=None, trg_e=None):
    pv, tv, _, _, _ = _aug_tables(pred_vertices, trg_vertices)
    in_maps = _prep_inputs(pred_vertices, trg_vertices)
    results = run_device(in_maps)
    return _exact_indices(results, pv, tv)


# revision 13
# speedup vs baseline: 9.0927x; 1.1234x over previous
"""Chamfer loss (two 16384x16384 1-NN searches + gathered MSE) on 8 Trainium2
cores.

Device (per core; queries sharded 8-way, 2048 per core per direction):
  - PE: score S[i,j] = q_i . r_j - |r_j|^2/2 = -(d(i,j) - |q_i|^2)/2 via ONE
    K=11 fp16 matmul per 512-col tile: augmented rows [qh4, qh4, ql3] x
    [rh4, rl4, rh3] implement the hi/lo split qh.rh + qh.rl + ql.rh, so the
    fp16 input rounding error (~3e-5) stays below the quantizer step.
    argmax_j S = argmin_j dist.
  - DVE tensor_reduce (max) folds each PSUM quad [128,2048] -> 128 slots
    directly (slot = j>>4 covers 16 consecutive j), so the fp32 scores never
    round through fp16 and no separate PSUM drain is needed.
  - ScalarE quantizes the folded row acc[1024] to int32: qi = rint(acc*4096).
  - Pool packs keys K = qi*32 + (slot&31); |K| <= 12M < 2^24 keeps every
    int32 op exact even through float ALU paths.
  - DVE reduces K per 32-slot class -> 32 winners/query/block; low 5 bits
    decode the winning slot. No InstMaxIndex anywhere (it costs ~256us/call
    on HW, ~8ms/core total in the previous version - the old bottleneck).

Host:
  - Decode 32 winner slots -> 32*16 = 512 candidate ids per query; exact f64
    re-scoring picks the true 1-NN (first-index tie-break). Measured on the
    fixed harness inputs: 19/32768 flips vs the fp32 reference argmin,
    loss rel-err 5.4e-05 (gate is 2e-2).
  - Gather e rows, squared-error means in f64 -> final f32 scalar.

Dispatch: run_bass_kernel_spmd's axon path rebuilds jax.jit(shard_map(...))
on every call, which re-runs neuronx_cc_hook -> bir_verify_and_optimise
(~300-450ms of client-side Python per call, scaling with instruction count).
_Runner builds the identical _bass_exec_p/shard_map wrapper once and caches
it, so steady-state calls are transfer + execute only.
"""
import sys

sys.path.insert(0, "/opt/trn_rl_repo")

import numpy as np

import concourse.bass as bass
import concourse.bacc as bacc
import concourse.mybir as mybir
from concourse.tile import TileContext

P = 128          # partitions / queries per block
V = 16384        # reference points
NCORES = 8
QPC = V // NCORES            # queries per core per direction (2048)
NBLK = QPC // P              # query blocks per core per direction (16)
NQUAD = 8                    # PSUM quads of 4 x 512 cols per block
KDIM = 11                    # augmented contraction: qh4+qh4+ql3
GRP = 16                     # j's per slot (slot = j >> 4)
NSLOT = V // GRP             # 1024 slots
LOC = 32                     # slots per class
NCLS = NSLOT // LOC          # 32 classes -> 32 winners/query/block
QSCALE = 4096.0              # quantizer: qi = rint(acc * 4096)
F16 = mybir.dt.float16
F32 = mybir.dt.float32
I32 = mybir.dt.int32
AX = mybir.AxisListType
OP = mybir.AluOpType

_CACHE = {}


def build(n_blocks=NBLK):
    nc = bacc.Bacc()
    qT = nc.dram_tensor("qT", [KDIM, 2 * QPC], F16, kind="ExternalInput")
    rT = nc.dram_tensor("rT", [KDIM, 2 * V], F16, kind="ExternalInput")
    # slot_out keeps the full-width shape for every n_blocks so that the
    # timing comparator (n_blocks=1) has identical host<->device transfers.
    slot_out = nc.dram_tensor(
        "slot_out", [P, 2 * NBLK * NCLS], F32, kind="ExternalOutput"
    )

    with TileContext(nc) as tc:
        with (
            tc.tile_pool(name="tab", bufs=1) as tab,
            tc.tile_pool(name="rtp", bufs=2) as rtp,
            tc.tile_pool(name="sb", bufs=2) as sb,
            tc.tile_pool(name="pk", bufs=2) as pk,
            tc.tile_pool(name="acc", bufs=1) as accp,
            tc.tile_pool(name="ps", bufs=2, space="PSUM") as ps,
        ):
            slotbuf = accp.tile([P, 2 * NBLK * NCLS], F32)
            if n_blocks < NBLK:
                nc.vector.memset(slotbuf[:], 0)

            qt = tab.tile([KDIM, 2 * QPC], F16)
            nc.sync.dma_start(out=qt[:], in_=qT[:])

            iota_l = tab.tile([P, NSLOT], I32)
            nc.gpsimd.iota(
                iota_l[:], pattern=[[0, NCLS], [1, LOC]], base=0,
                channel_multiplier=0,
            )
            iota_f = tab.tile([P, NSLOT], F32)
            nc.scalar.copy(iota_f[:], iota_l[:])


            for d in range(2):
                rt = rtp.tile([KDIM, V], F16, tag="rt")
                nc.sync.dma_start(out=rt[:], in_=rT[:, d * V : (d + 1) * V])
                for b in range(n_blocks):
                    lhsT = qt[:, d * QPC + b * P : d * QPC + (b + 1) * P]
                    acc = sb.tile([P, NSLOT], F32, tag="acc")
                    for g in range(NQUAD):
                        psq = ps.tile([P, 2048], F32, tag="psq")
                        for t in range(4):
                            nc.tensor.matmul(
                                out=psq[:, t * 512 : (t + 1) * 512],
                                lhsT=lhsT,
                                rhs=rt[:, (g * 4 + t) * 512 : (g * 4 + t + 1) * 512],
                                start=True,
                                stop=True,
                            )
                        nc.vector.tensor_reduce(
                            out=acc[:, g * P : (g + 1) * P],
                            in_=psq[:].rearrange("p (s g) -> p s g", g=GRP),
                            axis=AX.X,
                            op=OP.max,
                        )
                    # quantize: qi = rint(acc*4096) as int32 (exact on Act),
                    # then convert back to f32 scaled by 32 in one Act op.
                    # All keys are integer-valued f32 with |K| < 2^24, so
                    # every downstream float op is exact. (int32 max is
                    # unsupported on Pool and microcode-slow as a DVE
                    # tensor_reduce - ~194us/call.)
                    qi = pk.tile([P, NSLOT], I32, tag="qi")
                    nc.scalar.activation(
                        out=qi[:], in_=acc[:],
                        func=mybir.ActivationFunctionType.Copy, scale=QSCALE,
                    )
                    kq = pk.tile([P, NSLOT], F32, tag="kq")
                    nc.scalar.activation(
                        out=kq[:], in_=qi[:],
                        func=mybir.ActivationFunctionType.Copy, scale=float(LOC),
                    )
                    kk = pk.tile([P, NSLOT], F32, tag="kk")
                    nc.gpsimd.tensor_tensor(
                        out=kk[:], in0=kq[:], in1=iota_f[:], op=OP.add
                    )
                    # per-class argmax: binary f32 max tree on Pool
                    col = (d * n_blocks + b) * NCLS
                    cur, w = kk, LOC
                    while w > 2:
                        w //= 2
                        nxt = pk.tile([P, NCLS * w], F32, tag=f"tr{w}")
                        cv = cur[:].rearrange("p (c l) -> p c l", l=2 * w)
                        nc.vector.tensor_tensor(
                            out=nxt[:].rearrange("p (c l) -> p c l", l=w),
                            in0=cv[:, :, 0:w],
                            in1=cv[:, :, w : 2 * w],
                            op=OP.max,
                        )
                        cur = nxt
                    cv = cur[:].rearrange("p (c l) -> p c l", l=2)
                    nc.vector.tensor_tensor(
                        out=slotbuf[:, col : col + NCLS].rearrange(
                            "p (c l) -> p c l", l=1
                        ),
                        in0=cv[:, :, 0:1],
                        in1=cv[:, :, 1:2],
                        op=OP.max,
                    )
            nc.sync.dma_start(out=slot_out[:], in_=slotbuf[:])
    nc.compile()
    return nc


def _hilo(x):
    h = x.astype(np.float16)
    l = (x - h.astype(np.float32)).astype(np.float16)
    return h, l


def _aug_tables(pred_vertices, trg_vertices):
    pv = np.ascontiguousarray(pred_vertices[0]).astype(np.float32)  # [V,3]
    tv = np.ascontiguousarray(trg_vertices[0]).astype(np.float32)

    def aug_q_T(q):  # [11, Vq] fp16: [qh4, qh4, ql3]
        n = q.shape[0]
        qa = np.concatenate([q.T, np.ones((1, n), np.float32)], axis=0)  # [4,n]
        qh, ql = _hilo(qa)
        return np.concatenate([qh, qh, ql[:3]], axis=0)

    def aug_r_T(r):  # [11, V] fp16: [rh4, rl4, rh3]
        n2 = ((r * r).sum(1) * np.float32(0.5)).astype(np.float32)
        ra = np.concatenate([r.T, -n2[None, :]], axis=0)  # [4,V]
        rh, rl = _hilo(ra)
        return np.concatenate([rh, rl, rh[:3]], axis=0)

    # direction A: queries=tv, refs=pv;  direction B: queries=pv, refs=tv
    rT = np.ascontiguousarray(
        np.concatenate([aug_r_T(pv), aug_r_T(tv)], axis=1)
    )
    qT_A, qT_B = aug_q_T(tv), aug_q_T(pv)
    return pv, tv, rT, qT_A, qT_B


def _prep_inputs(pred_vertices, trg_vertices, pred_e=None, trg_e=None):
    _, _, rT, qT_A, qT_B = _aug_tables(pred_vertices, trg_vertices)
    in_maps = []
    for c in range(NCORES):
        sl = slice(c * QPC, (c + 1) * QPC)
        in_maps.append(
            {
                "qT": np.ascontiguousarray(
                    np.concatenate([qT_A[:, sl], qT_B[:, sl]], axis=1)
                ),
                "rT": rT,
            }
        )
    return in_maps


class _Runner:
    """Cached jit(shard_map(bass_exec)) dispatch for one compiled module.

    Mirrors run_bass_kernel_spmd's axon branch (bass2jax.run_bass_via_pjrt)
    but constructs the jitted callable once, so repeat calls skip the
    client-side re-trace / neuronx_cc_hook / BIR re-verification.
    """

    def __init__(self, nc):
        import jax
        from jax.sharding import Mesh, PartitionSpec
        from jax.experimental.shard_map import shard_map
        from concourse.bass2jax import (
            _bass_exec_p,
            install_neuronx_cc_hook,
            partition_id_tensor,
        )

        install_neuronx_cc_hook()
        partition_name = (
            nc.partition_id_tensor.name if nc.partition_id_tensor else None
        )
        in_names, out_names, out_avals, zero_shapes = [], [], [], []
        for alloc in nc.m.functions[0].allocations:
            if not isinstance(alloc, mybir.MemoryLocationSet):
                continue
            name = alloc.memorylocations[0].name
            if alloc.kind == "ExternalInput":
                if name != partition_name:
                    in_names.append(name)
            elif alloc.kind == "ExternalOutput":
                out_names.append(name)
                shape = tuple(alloc.tensor_shape)
                dtype = mybir.dt.np(alloc.dtype)
                out_avals.append(jax.core.ShapedArray(shape, dtype))
                zero_shapes.append((shape, dtype))
        n_params = len(in_names)
        all_names = list(in_names) + list(out_names)
        if partition_name is not None:
            all_names.append(partition_name)

        def _body(*args):
            operands = list(args)
            if partition_name is not None:
                operands.append(partition_id_tensor())
            outs = _bass_exec_p.bind(
                *operands,
                out_avals=tuple(out_avals),
                in_names=tuple(all_names),
                out_names=tuple(out_names),
                lowering_input_output_aliases=(),
                sim_require_finite=True,
                sim_require_nnan=True,
                nc=nc,
            )
            return tuple(outs)

        donate = tuple(range(n_params, n_params + len(out_avals)))
        devices = jax.devices()[:NCORES]
        mesh = Mesh(np.asarray(devices), ("core",))
        in_specs = (PartitionSpec("core"),) * (n_params + len(out_avals))
        out_specs = (PartitionSpec("core"),) * len(out_names)
        self._fn = jax.jit(
            shard_map(
                _body, mesh=mesh, in_specs=in_specs, out_specs=out_specs,
                check_rep=False,
            ),
            donate_argnums=donate,
            keep_unused=True,
        )
        self._in_names = in_names
        self._out_names = out_names
        self._out_avals = out_avals
        self._zero_shapes = zero_shapes

    def __call__(self, in_maps):
        concat_in = [
            np.concatenate([np.asarray(m[n]) for m in in_maps], axis=0)
            for n in self._in_names
        ]
        concat_zeros = [
            np.zeros((NCORES * s[0], *s[1:]), dt) for s, dt in self._zero_shapes
        ]
        out_arrs = self._fn(*concat_in, *concat_zeros)
        out_arrs = [np.asarray(o) for o in out_arrs]
        return [
            {
                name: out_arrs[i].reshape(NCORES, *self._out_avals[i].shape)[c]
                for i, name in enumerate(self._out_names)
            }
            for c in range(NCORES)
        ]


def get_runner(key="full", **build_kwargs):
    if key not in _CACHE:
        _CACHE[key] = _Runner(build(**build_kwargs))
    return _CACHE[key]


def run_device(in_maps):
    return get_runner()(in_maps)


_LOFF = np.arange(GRP, dtype=np.int64)


def _exact_indices(results, pv, tv):
    """Per-class winner keys -> 512 candidates/query -> exact f64 1-NN."""
    out = []
    cls_base = (np.arange(NCLS, dtype=np.int64) * LOC)[None, :]
    for d, (q, r) in enumerate([(tv, pv), (pv, tv)]):
        wins = np.empty((V, NCLS), np.int64)
        for c in range(NCORES):
            # [P, 2*NBLK*NCLS] f32, integer-valued keys |K| < 2^24
            so = np.rint(results[c]["slot_out"]).astype(np.int64)
            for b in range(NBLK):
                rows = slice(c * QPC + b * P, c * QPC + (b + 1) * P)
                col = (d * NBLK + b) * NCLS
                wins[rows] = so[:, col : col + NCLS]
        l = np.mod(wins, LOC)                      # floor-mod: l even for K<0
        slots = cls_base + l                       # [V, NCLS]
        cand = (slots[:, :, None] * GRP + _LOFF[None, None, :]).reshape(
            V, NCLS * GRP
        )
        q64 = q.astype(np.float64)
        r64 = r.astype(np.float64)
        idx = np.empty(V, np.int64)
        CH = 4096
        for s0 in range(0, V, CH):
            sl = slice(s0, s0 + CH)
            rc = r64[cand[sl]]                     # [CH,512,3]
            dd = ((rc - q64[sl][:, None, :]) ** 2).sum(axis=2)
            dmin = dd.min(axis=1)
            masked = np.where(dd <= dmin[:, None], cand[sl], 1 << 40)
            idx[sl] = masked.min(axis=1)
        out.append(idx)
    return out  # [idxA, idxB]


def kernel(pred_vertices, trg_vertices, pred_e, trg_e):
    pv, tv, _, _, _ = _aug_tables(pred_vertices, trg_vertices)
    in_maps = _prep_inputs(pred_vertices, trg_vertices)
    results = run_device(in_maps)
    idxA, idxB = _exact_indices(results, pv, tv)
    pe = np.ascontiguousarray(pred_e[0])
    te = np.ascontiguousarray(trg_e[0])
    lossA = ((te.astype(np.float64) - pe[idxA].astype(np.float64)) ** 2).sum() / (
        V * 3
    )
    lossB = ((pe.astype(np.float64) - te[idxB].astype(np.float64)) ** 2).sum() / (
        V * 3
    )
    return np.float32(lossA + lossB)


def kernel_indices(pred_vertices, trg_vertices, pred_e=None, trg_e=None):
    pv, tv, _, _, _ = _aug_tables(pred_vertices, trg_vertices)
    in_maps = _prep_inputs(pred_vertices, trg_vertices)
    results = run_device(in_maps)
    return _exact_indices(results, pv, tv)


# revision 15
# speedup vs baseline: 9.5347x; 1.0486x over previous
"""Chamfer loss (two 16384x16384 1-NN searches + gathered MSE) on 8 Trainium2
cores.

Device (per core; queries sharded 8-way, 2048 per core per direction):
  - PE: score S[i,j] = q_i . r_j - |r_j|^2/2 = -(d(i,j) - |q_i|^2)/2 via ONE
    K=11 fp16 matmul per 512-col tile: augmented rows [qh4, qh4, ql3] x
    [rh4, rl4, rh3] implement the hi/lo split qh.rh + qh.rl + ql.rh, so fp16
    input rounding error (~3e-5) stays below the quantizer step (2.4e-4).
    argmax_j S = argmin_j dist. fp16 runs 1 cycle/row (fp32r measured ~4x).
  - DVE folds each PSUM quad [128,2048] into acc[1024] by elementwise max
    (slot s = j mod 1024), quantizes with the fp32 magic-number trick
    (+2^23,-2^23 => exact rint), packs keys K = rint(acc*4096)*32 + (s>>5),
    and max-folds contiguous halves 1024->32: column c holds the winner of
    class {s : s mod 32 = c}; K mod 32 recovers s>>5. |K| <= 12M < 2^24, so
    every f32 op on keys is exact, and keys are unique per class (no ties).

    EVERYTHING after the matmul runs on DVE alone: on this part a dependent
    cross-engine hop costs ~50-200us when the consumer engine is idle, and a
    per-block Act->Pool->DVE pack chain measured 7-11ms total stall. A DVE
    tensor_reduce or InstMax/InstMaxIndex path is also out: those microcode
    at ~200-260us/call in-kernel. Plain 2D tensor_tensor/tensor_scalar ops
    issued back-to-back on one engine sidestep both failure modes.

Host:
  - Decode 32 winner slots -> 32*16 = 512 candidate ids per query; exact f64
    re-scoring picks the true 1-NN (first-index tie-break). Measured on the
    fixed harness inputs: 18/32768 flips vs the fp32 reference argmin,
    loss rel-err ~5e-05 (gate is 2e-2).
  - Gather e rows, squared-error means in f64 -> final f32 scalar.

Dispatch: run_bass_kernel_spmd's axon path rebuilds jax.jit(shard_map(...))
on every call, re-running neuronx_cc_hook -> bir_verify_and_optimise
(~300-450ms of client-side Python per call). _Runner builds the identical
_bass_exec_p/shard_map wrapper once and caches it, so steady-state calls are
transfer + execute only.
"""
import sys

sys.path.insert(0, "/opt/trn_rl_repo")

import numpy as np

import concourse.bass as bass
import concourse.bacc as bacc
import concourse.mybir as mybir
from concourse.tile import TileContext

P = 128          # partitions / queries per block
V = 16384        # reference points
NCORES = 8
QPC = V // NCORES            # queries per core per direction (2048)
NBLK = QPC // P              # query blocks per core per direction (16)
NQUAD = 8                    # PSUM quads of 4 x 512 cols per block
KDIM = 11                    # augmented contraction: qh4+qh4+ql3
NSLOT = 1024                 # slot = j mod 1024 (16 candidates per slot)
GRP = V // NSLOT             # 16 j's per slot
NCLS = 32                    # classes: c = s mod 32 (tree output columns)
LOC = NSLOT // NCLS          # 32 locals: l = s div 32
QSCALE = 4096.0              # quantizer step 1/4096
MAGIC = 8388608.0            # 2^23: x+MAGIC-MAGIC == rint(x) for |x|<2^22
F16 = mybir.dt.float16
F32 = mybir.dt.float32
I32 = mybir.dt.int32
OP = mybir.AluOpType

_CACHE = {}


def build(n_blocks=NBLK):
    nc = bacc.Bacc()
    qT = nc.dram_tensor("qT", [KDIM, 2 * QPC], F16, kind="ExternalInput")
    rT = nc.dram_tensor("rT", [KDIM, 2 * V], F16, kind="ExternalInput")
    # slot_out keeps the full-width shape for every n_blocks so that the
    # timing comparator (n_blocks=1) has identical host<->device transfers.
    slot_out = nc.dram_tensor(
        "slot_out", [P, 2 * NBLK * NCLS], F32, kind="ExternalOutput"
    )

    with TileContext(nc) as tc:
        with (
            tc.tile_pool(name="tab", bufs=1) as tab,
            tc.tile_pool(name="rtp", bufs=2) as rtp,
            tc.tile_pool(name="sb", bufs=2) as sb,
            tc.tile_pool(name="pk", bufs=2) as pk,
            tc.tile_pool(name="acc", bufs=1) as accp,
            tc.tile_pool(name="ps", bufs=2, space="PSUM") as ps,
        ):
            slotbuf = accp.tile([P, 2 * NBLK * NCLS], F32)
            if n_blocks < NBLK:
                nc.vector.memset(slotbuf[:], 0)

            qt = tab.tile([KDIM, 2 * QPC], F16)
            nc.sync.dma_start(out=qt[:], in_=qT[:])

            # iota_f[p, s] = s div 32 (the class-local index), directly as f32
            iota_f = tab.tile([P, NSLOT], F32)
            nc.gpsimd.iota(
                iota_f[:], pattern=[[1, LOC], [0, NCLS]], base=0,
                channel_multiplier=0, allow_small_or_imprecise_dtypes=True,
            )

            for d in range(2):
                rt = rtp.tile([KDIM, V], F16, tag="rt")
                nc.sync.dma_start(out=rt[:], in_=rT[:, d * V : (d + 1) * V])
                for b in range(n_blocks):
                    lhsT = qt[:, d * QPC + b * P : d * QPC + (b + 1) * P]
                    acc = sb.tile([P, NSLOT], F32, tag="acc")
                    for g in range(NQUAD):
                        psq = ps.tile([P, 2048], F32, tag="psq")
                        for t in range(4):
                            nc.tensor.matmul(
                                out=psq[:, t * 512 : (t + 1) * 512],
                                lhsT=lhsT,
                                rhs=rt[:, (g * 4 + t) * 512 : (g * 4 + t + 1) * 512],
                                start=True,
                                stop=True,
                            )
                        if g == 0:
                            nc.vector.tensor_copy(acc[:], psq[:, 0:NSLOT])
                        else:
                            nc.vector.tensor_tensor(
                                out=acc[:], in0=acc[:], in1=psq[:, 0:NSLOT],
                                op=OP.max,
                            )
                        nc.vector.tensor_tensor(
                            out=acc[:], in0=acc[:], in1=psq[:, NSLOT:2048],
                            op=OP.max,
                        )
                    # quantize+pack, all f32, all DVE:
                    #   qr = acc*4096 + 2^23          (integer-rounded + bias)
                    #   ka = (qr - 2^23) * 32         (= rint(acc*4096)*32)
                    #   kk = ka + iota_f              (+ class-local index)
                    qr = pk.tile([P, NSLOT], F32, tag="qr")
                    nc.vector.tensor_scalar(
                        out=qr[:], in0=acc[:], scalar1=QSCALE, scalar2=MAGIC,
                        op0=OP.mult, op1=OP.add,
                    )
                    ka = pk.tile([P, NSLOT], F32, tag="ka")
                    nc.vector.tensor_scalar(
                        out=ka[:], in0=qr[:], scalar1=-MAGIC, scalar2=float(LOC),
                        op0=OP.add, op1=OP.mult,
                    )
                    kk = pk.tile([P, NSLOT], F32, tag="kk")
                    nc.vector.tensor_tensor(
                        out=kk[:], in0=ka[:], in1=iota_f[:], op=OP.add
                    )
                    # per-class winner: contiguous-half max folds 1024 -> 32
                    col = (d * NBLK + b) * NCLS
                    cur, w = kk, NSLOT // 2
                    while w > NCLS:
                        nxt = pk.tile([P, w], F32, tag=f"t{w}")
                        nc.vector.tensor_tensor(
                            out=nxt[:], in0=cur[:, 0:w], in1=cur[:, w : 2 * w],
                            op=OP.max,
                        )
                        cur, w = nxt, w // 2
                    nc.vector.tensor_tensor(
                        out=slotbuf[:, col : col + NCLS],
                        in0=cur[:, 0:NCLS], in1=cur[:, NCLS : 2 * NCLS],
                        op=OP.max,
                    )
            nc.sync.dma_start(out=slot_out[:], in_=slotbuf[:])
    nc.compile()
    return nc


def _hilo(x):
    h = x.astype(np.float16)
    l = (x - h.astype(np.float32)).astype(np.float16)
    return h, l


def _aug_tables(pred_vertices, trg_vertices):
    pv = np.ascontiguousarray(pred_vertices[0]).astype(np.float32)  # [V,3]
    tv = np.ascontiguousarray(trg_vertices[0]).astype(np.float32)

    def aug_q_T(q):  # [11, Vq] fp16: [qh4, qh4, ql3]
        n = q.shape[0]
        qa = np.concatenate([q.T, np.ones((1, n), np.float32)], axis=0)  # [4,n]
        qh, ql = _hilo(qa)
        return np.concatenate([qh, qh, ql[:3]], axis=0)

    def aug_r_T(r):  # [11, V] fp16: [rh4, rl4, rh3]
        n2 = ((r * r).sum(1) * np.float32(0.5)).astype(np.float32)
        ra = np.concatenate([r.T, -n2[None, :]], axis=0)  # [4,V]
        rh, rl = _hilo(ra)
        return np.concatenate([rh, rl, rh[:3]], axis=0)

    # direction A: queries=tv, refs=pv;  direction B: queries=pv, refs=tv
    rT = np.ascontiguousarray(
        np.concatenate([aug_r_T(pv), aug_r_T(tv)], axis=1)
    )
    qT_A, qT_B = aug_q_T(tv), aug_q_T(pv)
    return pv, tv, rT, qT_A, qT_B


def _prep_inputs(pred_vertices, trg_vertices, pred_e=None, trg_e=None):
    _, _, rT, qT_A, qT_B = _aug_tables(pred_vertices, trg_vertices)
    in_maps = []
    for c in range(NCORES):
        sl = slice(c * QPC, (c + 1) * QPC)
        in_maps.append(
            {
                "qT": np.ascontiguousarray(
                    np.concatenate([qT_A[:, sl], qT_B[:, sl]], axis=1)
                ),
                "rT": rT,
            }
        )
    return in_maps


class _Runner:
    """Cached jit(shard_map(bass_exec)) dispatch for one compiled module.

    Mirrors run_bass_kernel_spmd's axon branch (bass2jax.run_bass_via_pjrt)
    but constructs the jitted callable once, so repeat calls skip the
    client-side re-trace / neuronx_cc_hook / BIR re-verification.
    """

    def __init__(self, nc):
        import jax
        from jax.sharding import Mesh, PartitionSpec
        from jax.experimental.shard_map import shard_map
        from concourse.bass2jax import (
            _bass_exec_p,
            install_neuronx_cc_hook,
            partition_id_tensor,
        )

        install_neuronx_cc_hook()
        partition_name = (
            nc.partition_id_tensor.name if nc.partition_id_tensor else None
        )
        in_names, out_names, out_avals, zero_shapes = [], [], [], []
        for alloc in nc.m.functions[0].allocations:
            if not isinstance(alloc, mybir.MemoryLocationSet):
                continue
            name = alloc.memorylocations[0].name
            if alloc.kind == "ExternalInput":
                if name != partition_name:
                    in_names.append(name)
            elif alloc.kind == "ExternalOutput":
                out_names.append(name)
                shape = tuple(alloc.tensor_shape)
                dtype = mybir.dt.np(alloc.dtype)
                out_avals.append(jax.core.ShapedArray(shape, dtype))
                zero_shapes.append((shape, dtype))
        n_params = len(in_names)
        all_names = list(in_names) + list(out_names)
        if partition_name is not None:
            all_names.append(partition_name)

        def _body(*args):
            operands = list(args)
            if partition_name is not None:
                operands.append(partition_id_tensor())
            outs = _bass_exec_p.bind(
                *operands,
                out_avals=tuple(out_avals),
                in_names=tuple(all_names),
                out_names=tuple(out_names),
                lowering_input_output_aliases=(),
                sim_require_finite=True,
                sim_require_nnan=True,
                nc=nc,
            )
            return tuple(outs)

        donate = tuple(range(n_params, n_params + len(out_avals)))
        devices = jax.devices()[:NCORES]
        mesh = Mesh(np.asarray(devices), ("core",))
        in_specs = (PartitionSpec("core"),) * (n_params + len(out_avals))
        out_specs = (PartitionSpec("core"),) * len(out_names)
        self._fn = jax.jit(
            shard_map(
                _body, mesh=mesh, in_specs=in_specs, out_specs=out_specs,
                check_rep=False,
            ),
            donate_argnums=donate,
            keep_unused=True,
        )
        self._in_names = in_names
        self._out_names = out_names
        self._out_avals = out_avals
        self._zero_shapes = zero_shapes

    def __call__(self, in_maps):
        concat_in = [
            np.concatenate([np.asarray(m[n]) for m in in_maps], axis=0)
            for n in self._in_names
        ]
        concat_zeros = [
            np.zeros((NCORES * s[0], *s[1:]), dt) for s, dt in self._zero_shapes
        ]
        out_arrs = self._fn(*concat_in, *concat_zeros)
        out_arrs = [np.asarray(o) for o in out_arrs]
        return [
            {
                name: out_arrs[i].reshape(NCORES, *self._out_avals[i].shape)[c]
                for i, name in enumerate(self._out_names)
            }
            for c in range(NCORES)
        ]


def get_runner(key="full", **build_kwargs):
    if key not in _CACHE:
        _CACHE[key] = _Runner(build(**build_kwargs))
    return _CACHE[key]


def run_device(in_maps):
    return get_runner()(in_maps)


def _exact_indices(results, pv, tv):
    """Per-class winner keys -> 512 candidates/query -> exact f64 1-NN."""
    out = []
    for d, (q, r) in enumerate([(tv, pv), (pv, tv)]):
        wins = np.empty((V, NCLS), np.int64)
        for c in range(NCORES):
            # [P, 2*NBLK*NCLS] f32, integer-valued keys |K| < 2^24
            so = np.rint(results[c]["slot_out"]).astype(np.int64)
            for b in range(NBLK):
                rows = slice(c * QPC + b * P, c * QPC + (b + 1) * P)
                col = (d * NBLK + b) * NCLS
                wins[rows] = so[:, col : col + NCLS]
        l = np.mod(wins, LOC)                        # l = s div 32 (floor-mod)
        slots = l * NCLS + np.arange(NCLS)[None, :]  # s = l*32 + c
        cand = (
            slots[:, :, None] + (np.arange(GRP) * NSLOT)[None, None, :]
        ).reshape(V, NCLS * GRP)                     # j = s + 1024*m
        q64 = q.astype(np.float64)
        r64 = r.astype(np.float64)
        idx = np.empty(V, np.int64)
        CH = 4096
        for s0 in range(0, V, CH):
            sl = slice(s0, s0 + CH)
            rc = r64[cand[sl]]                       # [CH,512,3]
            dd = ((rc - q64[sl][:, None, :]) ** 2).sum(axis=2)
            dmin = dd.min(axis=1)
            masked = np.where(dd <= dmin[:, None], cand[sl], 1 << 40)
            idx[sl] = masked.min(axis=1)
        out.append(idx)
    return out  # [idxA, idxB]


def kernel(pred_vertices, trg_vertices, pred_e, trg_e):
    pv, tv, _, _, _ = _aug_tables(pred_vertices, trg_vertices)
    in_maps = _prep_inputs(pred_vertices, trg_vertices)
    results = run_device(in_maps)
    idxA, idxB = _exact_indices(results, pv, tv)
    pe = np.ascontiguousarray(pred_e[0])
    te = np.ascontiguousarray(trg_e[0])
    lossA = ((te.astype(np.float64) - pe[idxA].astype(np.float64)) ** 2).sum() / (
        V * 3
    )
    lossB = ((pe.astype(np.float64) - te[idxB].astype(np.float64)) ** 2).sum() / (
        V * 3
    )
    return np.float32(lossA + lossB)


def kernel_indices(pred_vertices, trg_vertices, pred_e=None, trg_e=None):
    pv, tv, _, _, _ = _aug_tables(pred_vertices, trg_vertices)
    in_maps = _prep_inputs(pred_vertices, trg_vertices)
    results = run_device(in_maps)
    return _exact_indices(results, pv, tv)
